# revision 20
# baseline (speedup 1.0000x reference)
"""Trainium2 Bass kernel for nn_ABCFramework_17755394802208.

Conv backbone (6x 3x3 SAME convs + 2 maxpools, 256^2 -> 64^2) feeding full
self-attention over N=4096 tokens with a Swin-style relative-position bias.

Sharding: 8 cores = (batch b in {0,1}) x (head h in {0..3}); each core runs the
conv backbone for its batch, projects q/k/v for its head, and computes full
attention for its (b, h). Output slices are gathered/reassembled on host.

Conv formulation: each matmul processes G image rows at once. The moving tile
holds rows (ci, g') x (strip s, padded col xp) where g' in [0, G+2) spans the
G rows plus one halo row on each side -- ONE load DMA per input channel. The
stationary weights are zero-padded block-diagonal [Cin*(G+2), Cout*G], one per
(ky, kx) tap, accumulated over the 9 taps in PSUM (the ky shift lives in the
stationary's diagonal offset, so all 9 matmuls share the same moving slice
modulo the kx column offset). conv1 (Cin=1) uses the cheaper 3-tap variant
with rows (ky, g). All matmuls run in bf16.

Attention: logits S^T = K_c^T Q_j into a 3-bank PSUM tile (3 key-chunks), one
wide exp on the Scalar engine per 3 chunks, then the relative-position bias is
applied multiplicatively on the Vector engine: exp(s+b) = exp(s) * EB where
EB = exp(bias atlas) is computed once on-chip. AV accumulates in PSUM with an
extra ones-row in V giving the softmax denominators; the final divide uses a
gpsimd partition_broadcast of the reciprocal row.
"""
import sys

sys.path.insert(0, '/opt/trn_rl_repo')

import numpy as np

try:
    from ml_dtypes import bfloat16 as BF16_NP
except ImportError:  # pragma: no cover
    import jax.numpy as _jnp
    BF16_NP = _jnp.bfloat16

NUM_HEADS = 4
DIM_HEAD = 64
TABLE_M = 160
B = 2
N = 4096          # tokens (64 x 64)
GRID = 64
NCH = 32          # m-chunks of 128 keys
NQC = 8           # n-chunks of 512 queries
CF = 8192         # 127 * 64 compact-table window length

# conv layer configs: (Cin, Cout, H, W, G rows-per-group)
# layer 0 uses the 3-tap scheme (rows 3*G), layers 1+ the 9-tap (Cin*(G+2))
CONVS = [(1, 3, 256, 256, 42), (3, 3, 256, 256, 40),
         (3, 6, 128, 128, 21), (6, 6, 128, 128, 19),
         (6, 9, 64, 64, 14), (9, 9, 64, 64, 12)]

# plane name -> (C, H, W, slack rows below the bottom guard)
PLANES = {
    'x':  (1, 256, 256, 38),
    'm1': (3, 256, 256, 24),
    'm2': (3, 256, 256, 0),
    'p1': (3, 128, 128, 19),
    'm3': (6, 128, 128, 5),
    'm4': (6, 128, 128, 0),
    'p2': (6, 64, 64, 6),
    'm5': (9, 64, 64, 8),
}


def _lay(H, W, slack):
    Wp = W + 2
    Goff = Wp + 1
    rows = H + 2 + slack
    return Wp, Goff, rows * Wp + 2 * Goff  # Wp, guard offset, buflen


def _chunks(Sf, R, W):
    ns_max = 512 // W
    out = []
    s = 0
    while s < Sf:
        ns = min(ns_max, Sf - s)
        out.append((s, ns))
        s += ns
    if R:
        out.append((Sf, 1))
    return out


_BUILD_CACHE = {}


def _build():
    if 'nc' in _BUILD_CACHE:
        return _BUILD_CACHE['nc']
    import concourse.bass as bass
    import concourse.mybir as mybir
    import concourse.tile as tile
    from concourse import bacc

    F32 = mybir.dt.float32
    BF16 = mybir.dt.bfloat16
    AF = mybir.ActivationFunctionType
    ALU = mybir.AluOpType

    nc = bacc.Bacc("TRN2", target_bir_lowering=False, debug=False, num_devices=8)

    # ---- external inputs (per-core shards prepared on host) ----
    _, _, BLX = _lay(*PLANES['x'][1:])
    x_d = nc.dram_tensor("x", [1, BLX], BF16, kind="ExternalInput")
    wk_d, bx_d = [], []
    for i, (ci, co, _, _, G) in enumerate(CONVS):
        if i == 0:
            rows, cols, ntap = 3 * G, co * G, 3
        else:
            rows, cols, ntap = ci * (G + 2), co * G, 9
        wk_d.append([nc.dram_tensor(f"w{i}_{t}", [rows, cols], BF16,
                                    kind="ExternalInput") for t in range(ntap)])
        bx_d.append(nc.dram_tensor(f"bx{i}", [cols], F32, kind="ExternalInput"))
    wq_d = nc.dram_tensor("wq", [9, 64], BF16, kind="ExternalInput")
    wkk_d = nc.dram_tensor("wk", [9, 64], BF16, kind="ExternalInput")
    wv_d = nc.dram_tensor("wv", [9, 64], BF16, kind="ExternalInput")
    wa_d = nc.dram_tensor("watlas", [128, CF], BF16, kind="ExternalInput")
    out_d = nc.dram_tensor("out", [64, N], F32, kind="ExternalOutput")

    with tile.TileContext(nc) as tc:
        with tc.tile_pool(name="const", bufs=1) as const, \
             tc.tile_pool(name="work", bufs=2) as work, \
             tc.tile_pool(name="dram", bufs=1, space="DRAM") as dram:

            # ---------------- constant tiles (loads issued lazily) ----------
            wkt, bxt = [], []
            for i, (ci, co, _, _, G) in enumerate(CONVS):
                if i == 0:
                    rows, cols, ntap = 3 * G, co * G, 3
                else:
                    rows, cols, ntap = ci * (G + 2), co * G, 9
                wkt.append([const.tile([rows, cols], BF16, tag=f"w{i}_{t}",
                                       name=f"w{i}_{t}") for t in range(ntap)])
                bxt.append(const.tile([cols, 1], F32, tag=f"bx{i}",
                                      name=f"bx{i}"))

            zeros_bf = const.tile([128, 512], BF16, tag="zbf")
            nc.vector.memset(zeros_bf, 0.0)

            # ---------------- DRAM planes + zero scratch ----------------
            geom, plane = {}, {}
            for nm, (C, H, Wd, slack) in PLANES.items():
                geom[nm] = _lay(H, Wd, slack)
                if nm != 'x':
                    plane[nm] = dram.tile([C, geom[nm][2]], BF16, tag=nm, name=nm)
            tok_d = dram.tile([9, N], BF16, tag="tok", name="tok")
            zs = dram.tile([1, 32768], BF16, tag="zs", name="zs")
            nc.sync.dma_start(
                out=bass.AP(tensor=zs.tensor, offset=0, ap=[[256, 128], [1, 256]]),
                in_=zeros_bf[0:128, 0:256])

            def zfill(tensor, offset, ap):
                total = 1
                for _, cnt in ap:
                    total *= cnt
                assert total <= 32768, total
                nc.sync.dma_start(
                    out=bass.AP(tensor=tensor, offset=offset, ap=ap),
                    in_=bass.AP(tensor=zs.tensor, offset=0, ap=[[1, total]]))

            # initial guards: top strip + bottom strip (guard row + slack) for
            # every on-chip plane; conv writes cover the column guards (padded
            # width), pool-output planes also need column guards
            for nm in ['m1', 'm2', 'p1', 'm3', 'm4', 'p2', 'm5']:
                C, H, Wd, slack = PLANES[nm]
                Wp, Goff, bl = geom[nm]
                t = plane[nm].tensor
                zfill(t, 0, [[bl, C], [1, Goff + Wp]])
                zfill(t, Goff + (H + 1) * Wp,
                      [[bl, C], [1, (slack + 1) * Wp + Goff]])
                if nm in ('p1', 'p2'):
                    zfill(t, Goff + Wp, [[bl, C], [Wp, H + 1 + slack]])
                    zfill(t, Goff + 2 * Wp - 1, [[bl, C], [Wp, H + 1 + slack]])

            # ---------------- conv backbone ----------------
            def conv_layer(li, in_nm, out_nm):
                Cin, Cout, H, Wd, G = CONVS[li]
                Wp, Goff, bl = geom[in_nm]
                Sf, R = H // G, H % G
                S_tot = Sf + (1 if R else 0)
                mode3 = (li == 0)
                rows = 3 * G if mode3 else Cin * (G + 2)
                cols = Cout * G
                in_t = x_d if in_nm == 'x' else plane[in_nm].tensor
                ceng = [nc.gpsimd, nc.sync, nc.scalar][li % 3]
                for t, wt in enumerate(wkt[li]):
                    ceng.dma_start(out=wt, in_=wk_d[li][t][:, :])
                ceng.dma_start(out=bxt[li], in_=bx_d[li][:, None])

                mov = work.tile([rows, S_tot, Wp], BF16, tag="mov", bufs=2,
                                name=f"mov{li}")
                lengs = [nc.sync, nc.scalar]
                if mode3:
                    for ky in range(3):
                        src = bass.AP(tensor=in_t, offset=Goff + ky * Wp,
                                      ap=[[Wp, G], [G * Wp, S_tot], [1, Wp]])
                        lengs[ky % 2].dma_start(
                            out=mov[ky * G:(ky + 1) * G, :, :], in_=src)
                else:
                    for ci in range(Cin):
                        src = bass.AP(tensor=in_t, offset=ci * bl + Goff,
                                      ap=[[Wp, G + 2], [G * Wp, S_tot], [1, Wp]])
                        pb = ci * (G + 2)
                        lengs[ci % 2].dma_start(
                            out=mov[pb:pb + G + 2, :, :], in_=src)

                Wdo = Wd + 2  # write padded width so edge guards ride along
                out_t = work.tile([cols, S_tot, Wdo], BF16, tag="out", bufs=2,
                                  name=f"out{li}")
                nc.vector.memset(out_t[:, :, 0:1], 0.0)
                nc.vector.memset(out_t[:, :, Wdo - 1:Wdo], 0.0)

                chs = _chunks(Sf, R, Wd)
                taps = wkt[li]  # mode3: [ky] with kx folded; else [(ky,kx)]
                for g0 in range(0, len(chs), 3):
                    grp = chs[g0:g0 + 3]
                    pts = [psc.tile([cols, 512], F32, tag="cps",
                                    name=f"cps{li}_{g0}_{gi}")
                           for gi in range(len(grp))]
                    if mode3:
                        for kx in range(3):
                            for pt, (s0, ns) in zip(pts, grp):
                                nc.tensor.matmul(
                                    pt[:, 0:ns * Wd], taps[kx],
                                    mov[:, s0:s0 + ns, kx:kx + Wd],
                                    start=(kx == 0), stop=(kx == 2))
                    else:
                        for t in range(9):
                            ky, kx = t // 3, t % 3
                            for pt, (s0, ns) in zip(pts, grp):
                                nc.tensor.matmul(
                                    pt[:, 0:ns * Wd], taps[t],
                                    mov[:, s0:s0 + ns, kx:kx + Wd],
                                    start=(t == 0), stop=(t == 8))
                    for pt, (s0, ns) in zip(pts, grp):
                        nc.vector.scalar_tensor_tensor(
                            out=out_t[:, s0:s0 + ns, 1:1 + Wd],
                            in0=pt[:, 0:ns * Wd],
                            scalar=bxt[li], in1=zeros_bf[0:cols, 0:ns * Wd],
                            op0=ALU.add, op1=ALU.max)

                wengs = [nc.gpsimd, nc.sync]
                if out_nm == 'tok':
                    for co in range(Cout):
                        dst = bass.AP(tensor=tok_d.tensor, offset=co * N,
                                      ap=[[64, G], [G * 64, Sf], [1, 64]])
                        wengs[co % 2].dma_start(
                            out=dst,
                            in_=out_t[co * G:(co + 1) * G, 0:Sf, 1:1 + Wd])
                        if R:
                            dst = bass.AP(tensor=tok_d.tensor,
                                          offset=co * N + Sf * G * 64,
                                          ap=[[64, R], [1, 64]])
                            wengs[(co + 1) % 2].dma_start(
                                out=dst,
                                in_=out_t[co * G:co * G + R, Sf, 1:1 + Wd])
                    return
                Wpo, Goffo, blo = geom[out_nm]
                ot = plane[out_nm].tensor
                # one write per output channel (padded width covers the column
                # guards) + an exact R-row write for the partial strip, so the
                # pre-zeroed bottom guard/slack rows are never clobbered
                for co in range(Cout):
                    dst = bass.AP(tensor=ot, offset=co * blo + Goffo + Wpo,
                                  ap=[[Wpo, G], [G * Wpo, Sf], [1, Wdo]])
                    wengs[co % 2].dma_start(
                        out=dst, in_=out_t[co * G:(co + 1) * G, 0:Sf, :])
                    if R:
                        dst = bass.AP(tensor=ot,
                                      offset=co * blo + Goffo + (Sf * G + 1) * Wpo,
                                      ap=[[Wpo, R], [1, Wdo]])
                        wengs[(co + 1) % 2].dma_start(
                            out=dst, in_=out_t[co * G:co * G + R, Sf, :])

            def pool_layer(in_nm, out_nm):
                C, H, Wd, _ = PLANES[in_nm]
                Wp, Goff, bl = geom[in_nm]
                H2, W2 = H // 2, Wd // 2
                Wp2, Goff2, bl2 = geom[out_nm]
                it, ot = plane[in_nm].tensor, plane[out_nm].tensor
                # one load/store per channel: output rows on partitions (H2<=128)
                qs = [nc.sync, nc.scalar, nc.gpsimd]
                for c in range(C):
                    t3 = work.tile([128, 2, Wd], BF16, tag="pool", bufs=3,
                                   name=f"pool_{in_nm}_{c}")
                    src = bass.AP(tensor=it,
                                  offset=c * bl + Goff + Wp + 1,
                                  ap=[[2 * Wp, H2], [Wp, 2], [1, Wd]])
                    qs[c % 3].dma_start(out=t3[0:H2, :, :], in_=src)
                    m1t = work.tile([128, 2, W2], BF16, tag="plw", bufs=2)
                    nc.vector.tensor_max(m1t[0:H2], t3[0:H2, :, 0::2],
                                         t3[0:H2, :, 1::2])
                    m2t = work.tile([128, W2], BF16, tag="plh", bufs=2)
                    nc.vector.tensor_max(m2t[0:H2], m1t[0:H2, 0, :],
                                         m1t[0:H2, 1, :])
                    dst = bass.AP(tensor=ot, offset=c * bl2 + Goff2 + Wp2 + 1,
                                  ap=[[Wp2, H2], [1, W2]])
                    qs[(c + 1) % 3].dma_start(out=dst, in_=m2t[0:H2, :])

            scope_conv = nc.named_scope("conv"); scope_conv.__enter__()
            with tc.tile_pool(name="psc", bufs=6, space="PSUM") as psc:
                conv_layer(0, 'x', 'm1')
                conv_layer(1, 'm1', 'm2')
                pool_layer('m2', 'p1')
                conv_layer(2, 'p1', 'm3')
                conv_layer(3, 'm3', 'm4')
                pool_layer('m4', 'p2')
                conv_layer(4, 'p2', 'm5')
                conv_layer(5, 'm5', 'tok')
            scope_conv.__exit__(None, None, None)

            # ---------------- tokens + q/k/v ----------------
            scope_qkv = nc.named_scope("qkv"); scope_qkv.__enter__()
            # attention constants load now, off the critical conv queues
            wq_t = const.tile([9, 64], BF16, tag="wq")
            wkk_t = const.tile([9, 64], BF16, tag="wkk")
            wv_t = const.tile([9, 64], BF16, tag="wv")
            nc.gpsimd.dma_start(out=wq_t, in_=wq_d[:, :])
            nc.gpsimd.dma_start(out=wkk_t, in_=wkk_d[:, :])
            nc.gpsimd.dma_start(out=wv_t, in_=wv_d[:, :])
            Wt = const.tile([128, CF], BF16, tag="W")
            nc.scalar.dma_start(out=Wt, in_=wa_d[:, :])
            EB = const.tile([128, CF], BF16, tag="EB")
            nc.scalar.activation(out=EB, in_=Wt, func=AF.Exp)

            tokT = const.tile([9, N], BF16, tag="tok")
            nc.sync.dma_start(out=tokT, in_=tok_d[:, :])

            qT = const.tile([64, N], BF16, tag="qT")
            kT = const.tile([64, N], BF16, tag="kT")
            v_sb = const.tile([128, NCH, 65], BF16, tag="v")
            nc.vector.memset(v_sb, 1.0)

            with tc.tile_pool(name="psq", bufs=2, space="PSUM") as psq:
                for j in range(NQC):
                    ps_q = psq.tile([64, 512], F32, tag="qps")
                    nc.tensor.matmul(ps_q, wq_t, tokT[:, j * 512:(j + 1) * 512],
                                     start=True, stop=True)
                    nc.scalar.activation(out=qT[:, j * 512:(j + 1) * 512],
                                         in_=ps_q, func=AF.Copy,
                                         scale=float(DIM_HEAD) ** -0.5)
                    ps_k = psq.tile([64, 512], F32, tag="kps")
                    nc.tensor.matmul(ps_k, wkk_t, tokT[:, j * 512:(j + 1) * 512],
                                     start=True, stop=True)
                    nc.scalar.activation(out=kT[:, j * 512:(j + 1) * 512],
                                         in_=ps_k, func=AF.Copy)
                for c in range(NCH):
                    ps_v = psq.tile([128, 64], F32, tag="vps")
                    nc.tensor.matmul(ps_v, tokT[:, c * 128:(c + 1) * 128], wv_t,
                                     start=True, stop=True)
                    nc.vector.tensor_copy(v_sb[:, c, 0:64], ps_v)
            scope_qkv.__exit__(None, None, None)

            # ---------------- attention ----------------
            scope_attn = nc.named_scope("attn"); scope_attn.__enter__()
            with tc.tile_pool(name="pss", bufs=2, space="PSUM") as pss, \
                 tc.tile_pool(name="psa", bufs=2, space="PSUM") as psa:
                for j in range(NQC):
                    acc = psa.tile([65, 512], F32, tag="acc")
                    for cg in range(0, NCH, 3):
                        w = min(3, NCH - cg)
                        # S^T for chunks cg..cg+w-1, one 3-bank PSUM tile;
                        # exp over all w*512 columns in a single ACT op
                        s3 = pss.tile([128, 3, 512], F32, tag="s3")
                        for i in range(w):
                            c = cg + i
                            nc.tensor.matmul(s3[:, i, :],
                                             kT[:, c * 128:(c + 1) * 128],
                                             qT[:, j * 512:(j + 1) * 512],
                                             start=True, stop=True)
                        at3 = work.tile([128, 3, 512], BF16, tag="at", bufs=3)
                        nc.scalar.activation(out=at3[:, 0:w, :],
                                             in_=s3[:, 0:w, :], func=AF.Exp)
                        atb3 = work.tile([128, 3, 512], BF16, tag="atb", bufs=3)
                        for i in range(w):
                            c = cg + i
                            s0 = (8 * j - 2 * c + 63) * 64
                            nc.vector.tensor_mul(atb3[:, i, :], at3[:, i, :],
                                                 EB[:, s0:s0 + 512])
                        for i in range(w):
                            c = cg + i
                            nc.tensor.matmul(acc, v_sb[:, c, :], atb3[:, i, :],
                                             start=(c == 0),
                                             stop=(c == NCH - 1))
                    # epilogue: divide by the attention sums (row 64 of acc)
                    sums = work.tile([1, 512], F32, tag="sums", bufs=2)
                    nc.vector.tensor_copy(sums, acc[64:65, :])
                    rcp_f = work.tile([1, 512], F32, tag="rcpf", bufs=2)
                    nc.vector.reciprocal_approx_fast(out=rcp_f, in_=sums)
                    bc_sb = work.tile([64, 512], F32, tag="bcs", bufs=2)
                    nc.gpsimd.partition_broadcast(bc_sb, rcp_f)
                    res = work.tile([64, 512], F32, tag="res", bufs=2)
                    nc.vector.tensor_mul(res, acc[0:64, :], bc_sb)
                    nc.sync.dma_start(out=out_d[:, j * 512:(j + 1) * 512],
                                      in_=res)
            scope_attn.__exit__(None, None, None)

    nc.finalize()
    _BUILD_CACHE['nc'] = nc
    return nc


def _prep_inputs(inputs):
    """Build the 8 per-core input maps (layout/packing only)."""
    x = np.asarray(inputs['x'], dtype=np.float32)
    qkv_w = np.asarray(inputs['qkv_w'], dtype=np.float32)
    table = np.asarray(inputs['bias_table'], dtype=np.float32)

    Wp, Goff, BLX = _lay(*PLANES['x'][1:])
    rows_x = PLANES['x'][1] + 2 + PLANES['x'][3]
    xbufs = []
    for b in range(B):
        pad = np.zeros((rows_x, Wp), np.float32)
        pad[1:257, 1:257] = x[b, 0]
        buf = np.zeros((1, BLX), np.float32)
        buf[0, Goff:Goff + rows_x * Wp] = pad.reshape(-1)
        xbufs.append(buf.astype(BF16_NP))

    wks, bxs = [], []
    for i, (Cin, Cout, _, _, G) in enumerate(CONVS):
        w = np.asarray(inputs[f'conv{i + 1}_w'], dtype=np.float32)
        bias = np.asarray(inputs[f'conv{i + 1}_b'], dtype=np.float32)
        ar = np.arange(G)
        taps = []
        if i == 0:
            for kx in range(3):
                Wk = np.zeros((3 * G, Cout * G), np.float32)
                for ky in range(3):
                    for co in range(Cout):
                        Wk[ky * G + ar, co * G + ar] = w[co, 0, ky, kx]
                taps.append(Wk.astype(BF16_NP))
        else:
            for ky in range(3):
                for kx in range(3):
                    Wk = np.zeros((Cin * (G + 2), Cout * G), np.float32)
                    for ci in range(Cin):
                        for co in range(Cout):
                            Wk[ci * (G + 2) + ky + ar, co * G + ar] = \
                                w[co, ci, ky, kx]
                    taps.append(Wk.astype(BF16_NP))
        wks.append(taps)
        bxs.append(np.repeat(bias, G).astype(np.float32))

    atlases = []
    for h in range(NUM_HEADS):
        tab = table[:, h].reshape(2 * TABLE_M - 1, 2 * TABLE_M - 1)
        Ct = tab[96:96 + 127, 96:96 + 127]  # [127, 127]
        tmp = np.zeros((127, 128), np.float32)
        tmp[:, :127] = Ct
        cfbuf = np.zeros(191 + 16256 + 129, np.float32)
        cfbuf[191:191 + 16256] = tmp.reshape(-1)
        sw = np.lib.stride_tricks.sliding_window_view(cfbuf, 16256)
        p = np.arange(128)
        offs = 254 - (p % 64) - 128 * (p // 64)
        full = sw[offs]                                   # [128, 127*128]
        a2 = full.reshape(128, 127, 128)[:, :, 0:64].reshape(128, 127 * 64)
        atl = np.zeros((128, CF), np.float32)
        atl[:, :127 * 64] = a2
        atlases.append(atl.astype(BF16_NP))

    in_maps = []
    for core in range(8):
        b, h = core // 4, core % 4
        m = {"x": xbufs[b], "watlas": atlases[h]}
        for i in range(6):
            for t in range(len(wks[i])):
                m[f"w{i}_{t}"] = wks[i][t]
            m[f"bx{i}"] = bxs[i]
        m["wq"] = np.ascontiguousarray(
            qkv_w[h * 64:(h + 1) * 64, :].T).astype(BF16_NP)
        m["wk"] = np.ascontiguousarray(
            qkv_w[256 + h * 64:256 + (h + 1) * 64, :].T).astype(BF16_NP)
        m["wv"] = np.ascontiguousarray(
            qkv_w[512 + h * 64:512 + (h + 1) * 64, :].T).astype(BF16_NP)
        in_maps.append(m)
    return in_maps


def kernel(_trace=False, **inputs):
    from concourse.bass_utils import run_bass_kernel_spmd
    nc = _build()
    in_maps = _prep_inputs(inputs)
    import os
    tdir = os.environ.get("KTRACE_DIR")
    if tdir:
        os.makedirs(tdir, exist_ok=True)
    res = run_bass_kernel_spmd(nc, in_maps, core_ids=list(range(8)),
                               trace=_trace, tmpdir=tdir)
    if _trace:
        kernel.last_exec_ns = res.exec_time_ns
        kernel.last_results = res
    # assemble: core -> (b, h): [64(d), 4096(n)]
    O = np.stack([np.stack([res.results[b * 4 + h]["out"] for h in range(4)])
                  for b in range(B)])                      # [B, H, 64, N]
    out = O.transpose(0, 3, 1, 2).reshape(B, N, NUM_HEADS * DIM_HEAD)
    out = out.reshape(B, GRID, GRID, NUM_HEADS * DIM_HEAD)
    shift = int(np.asarray(inputs['window_size'])) // 2
    out = np.roll(out, shift=(-shift, -shift), axis=(1, 2))
    return out.astype(np.float32)


# revision 27
# speedup vs baseline: 1.0246x; 1.0246x over previous
"""Trainium2 Bass kernel for nn_ABCFramework_17755394802208.

Conv backbone (6x 3x3 SAME convs + 2 maxpools, 256^2 -> 64^2) feeding full
self-attention over N=4096 tokens with a Swin-style relative-position bias.

Sharding: 8 cores = (batch b in {0,1}) x (head h in {0..3}); each core runs the
conv backbone for its batch, projects q/k/v for its head, and computes full
attention for its (b, h). Output slices are gathered/reassembled on host.

Conv formulation: each matmul processes G image rows at once. The moving tile
holds rows (ci, g') x (strip s, padded col xp) where g' in [0, G+2) spans the
G rows plus one halo row on each side -- ONE load DMA per input channel. The
stationary weights are zero-padded block-diagonal [Cin*(G+2), Cout*G], one per
(ky, kx) tap, accumulated over the 9 taps in PSUM (the ky shift lives in the
stationary's diagonal offset, so all 9 matmuls share the same moving slice
modulo the kx column offset). conv1 (Cin=1) uses the cheaper 3-tap variant
with rows (ky, g). All matmuls run in bf16.

Attention: logits S^T = K_c^T Q_j into a 3-bank PSUM tile (3 key-chunks), one
wide exp on the Scalar engine per 3 chunks, then the relative-position bias is
applied multiplicatively on the Vector engine: exp(s+b) = exp(s) * EB where
EB = exp(bias atlas) is computed once on-chip. AV accumulates in PSUM with an
extra ones-row in V giving the softmax denominators; the final divide uses a
gpsimd partition_broadcast of the reciprocal row.
"""
import sys

sys.path.insert(0, '/opt/trn_rl_repo')

import numpy as np

try:
    from ml_dtypes import bfloat16 as BF16_NP
except ImportError:  # pragma: no cover
    import jax.numpy as _jnp
    BF16_NP = _jnp.bfloat16

NUM_HEADS = 4
DIM_HEAD = 64
TABLE_M = 160
B = 2
N = 4096          # tokens (64 x 64)
GRID = 64
NCH = 32          # m-chunks of 128 keys
NQC = 8           # n-chunks of 512 queries
CF = 8192         # 127 * 64 compact-table window length

# conv layer configs: (Cin, Cout, H, W, G rows-per-group)
# layer 0 uses the 3-tap scheme (rows 3*G), layers 1+ the 9-tap (Cin*(G+2))
CONVS = [(1, 3, 256, 256, 42), (3, 3, 256, 256, 40),
         (3, 6, 128, 128, 21), (6, 6, 128, 128, 19),
         (6, 9, 64, 64, 14), (9, 9, 64, 64, 12)]

# plane name -> (C, H, W, slack rows below the bottom guard)
PLANES = {
    'x':  (1, 256, 256, 38),
    'm1': (3, 256, 256, 24),
    'm2': (3, 256, 256, 0),
    'p1': (3, 128, 128, 19),
    'm3': (6, 128, 128, 5),
    'm4': (6, 128, 128, 0),
    'p2': (6, 64, 64, 6),
    'm5': (9, 64, 64, 8),
}


def _lay(H, W, slack):
    Wp = W + 2
    Goff = Wp + 1
    rows = H + 2 + slack
    return Wp, Goff, rows * Wp + 2 * Goff  # Wp, guard offset, buflen


def _chunks(Sf, R, W):
    ns_max = 512 // W
    out = []
    s = 0
    while s < Sf:
        ns = min(ns_max, Sf - s)
        out.append((s, ns))
        s += ns
    if R:
        out.append((Sf, 1))
    return out


_BUILD_CACHE = {}


def _build():
    if 'nc' in _BUILD_CACHE:
        return _BUILD_CACHE['nc']
    import concourse.bass as bass
    import concourse.mybir as mybir
    import concourse.tile as tile
    from concourse import bacc

    F32 = mybir.dt.float32
    BF16 = mybir.dt.bfloat16
    AF = mybir.ActivationFunctionType
    ALU = mybir.AluOpType

    nc = bacc.Bacc("TRN2", target_bir_lowering=False, debug=False, num_devices=8)

    # ---- external inputs (per-core shards prepared on host) ----
    _, _, BLX = _lay(*PLANES['x'][1:])
    x_d = nc.dram_tensor("x", [1, BLX], BF16, kind="ExternalInput")
    wk_d, bx_d = [], []
    for i, (ci, co, _, _, G) in enumerate(CONVS):
        if i == 0:
            rows, cols, ntap = 3 * G, co * G, 3
        else:
            rows, cols, ntap = ci * (G + 2), co * G, 9
        wk_d.append([nc.dram_tensor(f"w{i}_{t}", [rows, cols], BF16,
                                    kind="ExternalInput") for t in range(ntap)])
        bx_d.append(nc.dram_tensor(f"bx{i}", [cols], F32, kind="ExternalInput"))
    wq_d = nc.dram_tensor("wq", [9, 64], BF16, kind="ExternalInput")
    wkk_d = nc.dram_tensor("wk", [9, 64], BF16, kind="ExternalInput")
    wv_d = nc.dram_tensor("wv", [9, 64], BF16, kind="ExternalInput")
    wa_d = nc.dram_tensor("watlas", [128, CF], BF16, kind="ExternalInput")
    out_d = nc.dram_tensor("out", [64, N], F32, kind="ExternalOutput")

    with tile.TileContext(nc) as tc:
        with tc.tile_pool(name="const", bufs=1) as const, \
             tc.tile_pool(name="work", bufs=2) as work, \
             tc.tile_pool(name="dram", bufs=1, space="DRAM") as dram:

            # ---------------- constant tiles (loads issued lazily) ----------
            wkt, bxt = [], []
            for i, (ci, co, _, _, G) in enumerate(CONVS):
                if i == 0:
                    rows, cols, ntap = 3 * G, co * G, 3
                else:
                    rows, cols, ntap = ci * (G + 2), co * G, 9
                wkt.append([const.tile([rows, cols], BF16, tag=f"w{i}_{t}",
                                       name=f"w{i}_{t}") for t in range(ntap)])
                bxt.append(const.tile([cols, 1], F32, tag=f"bx{i}",
                                      name=f"bx{i}"))

            zeros_bf = const.tile([128, 512], BF16, tag="zbf")
            nc.vector.memset(zeros_bf, 0.0)

            # prefetch all conv constants on the scalar queue, in layer order,
            # so the sync queue only carries the latency-critical mov loads
            for i in range(6):
                for t, wt in enumerate(wkt[i]):
                    nc.scalar.dma_start(out=wt, in_=wk_d[i][t][:, :])
                nc.scalar.dma_start(out=bxt[i], in_=bx_d[i][:, None])
            wq_t = const.tile([9, 64], BF16, tag="wq")
            wkk_t = const.tile([9, 64], BF16, tag="wkk")
            wv_t = const.tile([9, 64], BF16, tag="wv")
            nc.scalar.dma_start(out=wq_t, in_=wq_d[:, :])
            nc.scalar.dma_start(out=wkk_t, in_=wkk_d[:, :])
            nc.scalar.dma_start(out=wv_t, in_=wv_d[:, :])
            Wt = const.tile([128, CF], BF16, tag="W")
            nc.scalar.dma_start(out=Wt, in_=wa_d[:, :])
            EB = const.tile([128, CF], BF16, tag="EB")
            nc.scalar.activation(out=EB, in_=Wt, func=AF.Exp)

            # ---------------- DRAM planes + zero scratch ----------------
            geom, plane = {}, {}
            for nm, (C, H, Wd, slack) in PLANES.items():
                geom[nm] = _lay(H, Wd, slack)
                if nm != 'x':
                    plane[nm] = dram.tile([C, geom[nm][2]], BF16, tag=nm, name=nm)
            tok_d = dram.tile([9, N], BF16, tag="tok", name="tok")
            zs = dram.tile([1, 32768], BF16, tag="zs", name="zs")
            nc.sync.dma_start(
                out=bass.AP(tensor=zs.tensor, offset=0, ap=[[256, 128], [1, 256]]),
                in_=zeros_bf[0:128, 0:256])

            def zfill(tensor, offset, ap):
                total = 1
                for _, cnt in ap:
                    total *= cnt
                assert total <= 32768, total
                nc.sync.dma_start(
                    out=bass.AP(tensor=tensor, offset=offset, ap=ap),
                    in_=bass.AP(tensor=zs.tensor, offset=0, ap=[[1, total]]))

            # initial guards: top strip + bottom strip (guard row + slack) for
            # every on-chip plane; conv writes cover the column guards (padded
            # width), pool-output planes also need column guards
            for nm in ['m1', 'm2', 'p1', 'm3', 'm4', 'p2', 'm5']:
                C, H, Wd, slack = PLANES[nm]
                Wp, Goff, bl = geom[nm]
                t = plane[nm].tensor
                zfill(t, 0, [[bl, C], [1, Goff + Wp]])
                zfill(t, Goff + (H + 1) * Wp,
                      [[bl, C], [1, (slack + 1) * Wp + Goff]])
                if nm in ('p1', 'p2'):
                    zfill(t, Goff + Wp, [[bl, C], [Wp, H + 1 + slack]])
                    zfill(t, Goff + 2 * Wp - 1, [[bl, C], [Wp, H + 1 + slack]])

            # ---------------- conv backbone ----------------
            def conv_layer(li, in_nm, out_nm):
                Cin, Cout, H, Wd, G = CONVS[li]
                Wp, Goff, bl = geom[in_nm]
                Sf, R = H // G, H % G
                S_tot = Sf + (1 if R else 0)
                mode3 = (li == 0)
                rows = 3 * G if mode3 else Cin * (G + 2)
                cols = Cout * G
                in_t = x_d if in_nm == 'x' else plane[in_nm].tensor

                mov = work.tile([rows, S_tot, Wp], BF16, tag="mov", bufs=2,
                                name=f"mov{li}")
                if mode3:
                    for ky in range(3):
                        src = bass.AP(tensor=in_t, offset=Goff + ky * Wp,
                                      ap=[[Wp, G], [G * Wp, S_tot], [1, Wp]])
                        nc.sync.dma_start(
                            out=mov[ky * G:(ky + 1) * G, :, :], in_=src)
                else:
                    for ci in range(Cin):
                        src = bass.AP(tensor=in_t, offset=ci * bl + Goff,
                                      ap=[[Wp, G + 2], [G * Wp, S_tot], [1, Wp]])
                        pb = ci * (G + 2)
                        nc.sync.dma_start(
                            out=mov[pb:pb + G + 2, :, :], in_=src)

                Wdo = Wd + 2  # write padded width so edge guards ride along
                out_t = work.tile([cols, S_tot, Wdo], BF16, tag="out", bufs=2,
                                  name=f"out{li}")
                nc.vector.memset(out_t[:, :, 0:1], 0.0)
                nc.vector.memset(out_t[:, :, Wdo - 1:Wdo], 0.0)

                chs = _chunks(Sf, R, Wd)
                taps = wkt[li]  # mode3: [ky] with kx folded; else [(ky,kx)]
                for g0 in range(0, len(chs), 3):
                    grp = chs[g0:g0 + 3]
                    pts = [psc.tile([cols, 512], F32, tag="cps",
                                    name=f"cps{li}_{g0}_{gi}")
                           for gi in range(len(grp))]
                    if mode3:
                        for kx in range(3):
                            for pt, (s0, ns) in zip(pts, grp):
                                nc.tensor.matmul(
                                    pt[:, 0:ns * Wd], taps[kx],
                                    mov[:, s0:s0 + ns, kx:kx + Wd],
                                    start=(kx == 0), stop=(kx == 2))
                    else:
                        for t in range(9):
                            ky, kx = t // 3, t % 3
                            for pt, (s0, ns) in zip(pts, grp):
                                nc.tensor.matmul(
                                    pt[:, 0:ns * Wd], taps[t],
                                    mov[:, s0:s0 + ns, kx:kx + Wd],
                                    start=(t == 0), stop=(t == 8))
                    for pt, (s0, ns) in zip(pts, grp):
                        nc.vector.scalar_tensor_tensor(
                            out=out_t[:, s0:s0 + ns, 1:1 + Wd],
                            in0=pt[:, 0:ns * Wd],
                            scalar=bxt[li], in1=zeros_bf[0:cols, 0:ns * Wd],
                            op0=ALU.add, op1=ALU.max)

                wengs = [nc.gpsimd, nc.sync]
                if out_nm == 'tok':
                    for co in range(Cout):
                        dst = bass.AP(tensor=tok_d.tensor, offset=co * N,
                                      ap=[[64, G], [G * 64, Sf], [1, 64]])
                        wengs[co % 2].dma_start(
                            out=dst,
                            in_=out_t[co * G:(co + 1) * G, 0:Sf, 1:1 + Wd])
                        if R:
                            dst = bass.AP(tensor=tok_d.tensor,
                                          offset=co * N + Sf * G * 64,
                                          ap=[[64, R], [1, 64]])
                            wengs[(co + 1) % 2].dma_start(
                                out=dst,
                                in_=out_t[co * G:co * G + R, Sf, 1:1 + Wd])
                    return
                Wpo, Goffo, blo = geom[out_nm]
                ot = plane[out_nm].tensor
                # one write per output channel (padded width covers the column
                # guards) + an exact R-row write for the partial strip, so the
                # pre-zeroed bottom guard/slack rows are never clobbered
                for co in range(Cout):
                    dst = bass.AP(tensor=ot, offset=co * blo + Goffo + Wpo,
                                  ap=[[Wpo, G], [G * Wpo, Sf], [1, Wdo]])
                    wengs[co % 2].dma_start(
                        out=dst, in_=out_t[co * G:(co + 1) * G, 0:Sf, :])
                    if R:
                        dst = bass.AP(tensor=ot,
                                      offset=co * blo + Goffo + (Sf * G + 1) * Wpo,
                                      ap=[[Wpo, R], [1, Wdo]])
                        wengs[(co + 1) % 2].dma_start(
                            out=dst, in_=out_t[co * G:co * G + R, Sf, :])

            def pool_layer(in_nm, out_nm):
                C, H, Wd, _ = PLANES[in_nm]
                Wp, Goff, bl = geom[in_nm]
                H2, W2 = H // 2, Wd // 2
                Wp2, Goff2, bl2 = geom[out_nm]
                it, ot = plane[in_nm].tensor, plane[out_nm].tensor
                # one load/store per channel: output rows on partitions (H2<=128)
                for c in range(C):
                    t3 = work.tile([128, 2, Wd], BF16, tag="pool", bufs=3,
                                   name=f"pool_{in_nm}_{c}")
                    src = bass.AP(tensor=it,
                                  offset=c * bl + Goff + Wp + 1,
                                  ap=[[2 * Wp, H2], [Wp, 2], [1, Wd]])
                    (nc.sync if c % 2 else nc.scalar).dma_start(
                        out=t3[0:H2, :, :], in_=src)
                    m1t = work.tile([128, 2, W2], BF16, tag="plw", bufs=2)
                    nc.vector.tensor_max(m1t[0:H2], t3[0:H2, :, 0::2],
                                         t3[0:H2, :, 1::2])
                    m2t = work.tile([128, W2], BF16, tag="plh", bufs=2)
                    nc.vector.tensor_max(m2t[0:H2], m1t[0:H2, 0, :],
                                         m1t[0:H2, 1, :])
                    dst = bass.AP(tensor=ot, offset=c * bl2 + Goff2 + Wp2 + 1,
                                  ap=[[Wp2, H2], [1, W2]])
                    (nc.gpsimd if c % 2 else nc.sync).dma_start(
                        out=dst, in_=m2t[0:H2, :])

            # dummy matmuls: fill PE idle between conv layers so the HAM
            # activity monitor holds the PE array at its warm (2.4 GHz) clock
            psw_cm = tc.tile_pool(name="psw", bufs=1, space="PSUM")
            psw = psw_cm.__enter__()
            warm_n = [0]

            def warm(n):
                for _ in range(n):
                    wt_ = psw.tile([128, 512], F32, tag="wm",
                                   name=f"wm{warm_n[0]}")
                    warm_n[0] += 1
                    nc.tensor.matmul(wt_, zeros_bf[0:64, 0:128],
                                     zeros_bf[0:64, 0:512],
                                     start=True, stop=True)

            scope_conv = nc.named_scope("conv"); scope_conv.__enter__()
            with tc.tile_pool(name="psc", bufs=6, space="PSUM") as psc:
                warm(28)
                conv_layer(0, 'x', 'm1')
                warm(24)
                conv_layer(1, 'm1', 'm2')
                warm(14)
                pool_layer('m2', 'p1')
                warm(14)
                conv_layer(2, 'p1', 'm3')
                warm(20)
                conv_layer(3, 'm3', 'm4')
                warm(14)
                pool_layer('m4', 'p2')
                warm(14)
                conv_layer(4, 'p2', 'm5')
                warm(20)
                conv_layer(5, 'm5', 'tok')
                warm(14)
            scope_conv.__exit__(None, None, None)

            # ---------------- tokens + q/k/v ----------------
            scope_qkv = nc.named_scope("qkv"); scope_qkv.__enter__()
            tokT = const.tile([9, N], BF16, tag="tok")
            nc.sync.dma_start(out=tokT, in_=tok_d[:, :])

            qT = const.tile([64, N], BF16, tag="qT")
            kT = const.tile([64, N], BF16, tag="kT")
            v_sb = const.tile([128, NCH, 65], BF16, tag="v")
            nc.vector.memset(v_sb, 1.0)

            with tc.tile_pool(name="psq", bufs=2, space="PSUM") as psq:
                for j in range(NQC):
                    ps_q = psq.tile([64, 512], F32, tag="qps")
                    nc.tensor.matmul(ps_q, wq_t, tokT[:, j * 512:(j + 1) * 512],
                                     start=True, stop=True)
                    nc.scalar.activation(out=qT[:, j * 512:(j + 1) * 512],
                                         in_=ps_q, func=AF.Copy,
                                         scale=float(DIM_HEAD) ** -0.5)
                    ps_k = psq.tile([64, 512], F32, tag="kps")
                    nc.tensor.matmul(ps_k, wkk_t, tokT[:, j * 512:(j + 1) * 512],
                                     start=True, stop=True)
                    nc.scalar.activation(out=kT[:, j * 512:(j + 1) * 512],
                                         in_=ps_k, func=AF.Copy)
                for c in range(NCH):
                    ps_v = psq.tile([128, 64], F32, tag="vps")
                    nc.tensor.matmul(ps_v, tokT[:, c * 128:(c + 1) * 128], wv_t,
                                     start=True, stop=True)
                    nc.vector.tensor_copy(v_sb[:, c, 0:64], ps_v)
                warm(10)
            scope_qkv.__exit__(None, None, None)
            psw_cm.__exit__(None, None, None)

            # ---------------- attention ----------------
            scope_attn = nc.named_scope("attn"); scope_attn.__enter__()
            with tc.tile_pool(name="pss", bufs=2, space="PSUM") as pss, \
                 tc.tile_pool(name="psa", bufs=2, space="PSUM") as psa:
                for j in range(NQC):
                    acc = psa.tile([65, 512], F32, tag="acc")
                    for cg in range(0, NCH, 3):
                        w = min(3, NCH - cg)
                        # S^T for chunks cg..cg+w-1, one 3-bank PSUM tile;
                        # exp over all w*512 columns in a single ACT op
                        s3 = pss.tile([128, 3, 512], F32, tag="s3")
                        for i in range(w):
                            c = cg + i
                            nc.tensor.matmul(s3[:, i, :],
                                             kT[:, c * 128:(c + 1) * 128],
                                             qT[:, j * 512:(j + 1) * 512],
                                             start=True, stop=True)
                        at3 = work.tile([128, 3, 512], BF16, tag="at", bufs=3)
                        nc.scalar.activation(out=at3[:, 0:w, :],
                                             in_=s3[:, 0:w, :], func=AF.Exp)
                        atb3 = work.tile([128, 3, 512], BF16, tag="atb", bufs=3)
                        for i in range(w):
                            c = cg + i
                            s0 = (8 * j - 2 * c + 63) * 64
                            nc.vector.tensor_mul(atb3[:, i, :], at3[:, i, :],
                                                 EB[:, s0:s0 + 512])
                        for i in range(w):
                            c = cg + i
                            nc.tensor.matmul(acc, v_sb[:, c, :], atb3[:, i, :],
                                             start=(c == 0),
                                             stop=(c == NCH - 1))
                    # epilogue: divide by the attention sums (row 64 of acc)
                    sums = work.tile([1, 512], F32, tag="sums", bufs=2)
                    nc.vector.tensor_copy(sums, acc[64:65, :])
                    rcp_f = work.tile([1, 512], F32, tag="rcpf", bufs=2)
                    nc.vector.reciprocal_approx_fast(out=rcp_f, in_=sums)
                    bc_sb = work.tile([64, 512], F32, tag="bcs", bufs=2)
                    nc.gpsimd.partition_broadcast(bc_sb, rcp_f)
                    res = work.tile([64, 512], F32, tag="res", bufs=2)
                    nc.vector.tensor_mul(res, acc[0:64, :], bc_sb)
                    nc.sync.dma_start(out=out_d[:, j * 512:(j + 1) * 512],
                                      in_=res)
            scope_attn.__exit__(None, None, None)

    nc.finalize()
    _BUILD_CACHE['nc'] = nc
    return nc


def _prep_inputs(inputs):
    """Build the 8 per-core input maps (layout/packing only)."""
    x = np.asarray(inputs['x'], dtype=np.float32)
    qkv_w = np.asarray(inputs['qkv_w'], dtype=np.float32)
    table = np.asarray(inputs['bias_table'], dtype=np.float32)

    Wp, Goff, BLX = _lay(*PLANES['x'][1:])
    rows_x = PLANES['x'][1] + 2 + PLANES['x'][3]
    xbufs = []
    for b in range(B):
        pad = np.zeros((rows_x, Wp), np.float32)
        pad[1:257, 1:257] = x[b, 0]
        buf = np.zeros((1, BLX), np.float32)
        buf[0, Goff:Goff + rows_x * Wp] = pad.reshape(-1)
        xbufs.append(buf.astype(BF16_NP))

    wks, bxs = [], []
    for i, (Cin, Cout, _, _, G) in enumerate(CONVS):
        w = np.asarray(inputs[f'conv{i + 1}_w'], dtype=np.float32)
        bias = np.asarray(inputs[f'conv{i + 1}_b'], dtype=np.float32)
        ar = np.arange(G)
        taps = []
        if i == 0:
            for kx in range(3):
                Wk = np.zeros((3 * G, Cout * G), np.float32)
                for ky in range(3):
                    for co in range(Cout):
                        Wk[ky * G + ar, co * G + ar] = w[co, 0, ky, kx]
                taps.append(Wk.astype(BF16_NP))
        else:
            for ky in range(3):
                for kx in range(3):
                    Wk = np.zeros((Cin * (G + 2), Cout * G), np.float32)
                    for ci in range(Cin):
                        for co in range(Cout):
                            Wk[ci * (G + 2) + ky + ar, co * G + ar] = \
                                w[co, ci, ky, kx]
                    taps.append(Wk.astype(BF16_NP))
        wks.append(taps)
        bxs.append(np.repeat(bias, G).astype(np.float32))

    atlases = []
    for h in range(NUM_HEADS):
        tab = table[:, h].reshape(2 * TABLE_M - 1, 2 * TABLE_M - 1)
        Ct = tab[96:96 + 127, 96:96 + 127]  # [127, 127]
        tmp = np.zeros((127, 128), np.float32)
        tmp[:, :127] = Ct
        cfbuf = np.zeros(191 + 16256 + 129, np.float32)
        cfbuf[191:191 + 16256] = tmp.reshape(-1)
        sw = np.lib.stride_tricks.sliding_window_view(cfbuf, 16256)
        p = np.arange(128)
        offs = 254 - (p % 64) - 128 * (p // 64)
        full = sw[offs]                                   # [128, 127*128]
        a2 = full.reshape(128, 127, 128)[:, :, 0:64].reshape(128, 127 * 64)
        atl = np.zeros((128, CF), np.float32)
        atl[:, :127 * 64] = a2
        atlases.append(atl.astype(BF16_NP))

    in_maps = []
    for core in range(8):
        b, h = core // 4, core % 4
        m = {"x": xbufs[b], "watlas": atlases[h]}
        for i in range(6):
            for t in range(len(wks[i])):
                m[f"w{i}_{t}"] = wks[i][t]
            m[f"bx{i}"] = bxs[i]
        m["wq"] = np.ascontiguousarray(
            qkv_w[h * 64:(h + 1) * 64, :].T).astype(BF16_NP)
        m["wk"] = np.ascontiguousarray(
            qkv_w[256 + h * 64:256 + (h + 1) * 64, :].T).astype(BF16_NP)
        m["wv"] = np.ascontiguousarray(
            qkv_w[512 + h * 64:512 + (h + 1) * 64, :].T).astype(BF16_NP)
        in_maps.append(m)
    return in_maps


def kernel(_trace=False, **inputs):
    from concourse.bass_utils import run_bass_kernel_spmd
    nc = _build()
    in_maps = _prep_inputs(inputs)
    import os
    tdir = os.environ.get("KTRACE_DIR")
    if tdir:
        os.makedirs(tdir, exist_ok=True)
    res = run_bass_kernel_spmd(nc, in_maps, core_ids=list(range(8)),
                               trace=_trace, tmpdir=tdir)
    if _trace:
        kernel.last_exec_ns = res.exec_time_ns
        kernel.last_results = res
    # assemble: core -> (b, h): [64(d), 4096(n)]
    O = np.stack([np.stack([res.results[b * 4 + h]["out"] for h in range(4)])
                  for b in range(B)])                      # [B, H, 64, N]
    out = O.transpose(0, 3, 1, 2).reshape(B, N, NUM_HEADS * DIM_HEAD)
    out = out.reshape(B, GRID, GRID, NUM_HEADS * DIM_HEAD)
    shift = int(np.asarray(inputs['window_size'])) // 2
    out = np.roll(out, shift=(-shift, -shift), axis=(1, 2))
    return out.astype(np.float32)


# revision 28
# speedup vs baseline: 1.2253x; 1.1959x over previous
"""Trainium2 Bass kernel for nn_ABCFramework_17755394802208.

Conv backbone (6x 3x3 SAME convs + 2 maxpools, 256^2 -> 64^2) feeding full
self-attention over N=4096 tokens with a Swin-style relative-position bias.

Sharding: 8 cores = (batch b in {0,1}) x (head h in {0..3}); each core runs the
conv backbone for its batch, projects q/k/v for its head, and computes full
attention for its (b, h). Output slices are gathered/reassembled on host.

Conv formulation: each matmul processes G image rows at once (G divides H, so
no partial strips). The moving tile holds rows (ci, g') x (strip s, padded col
xp) where g' in [0, G+2) spans the G rows plus a halo row on each side -- ONE
load DMA per input channel. The stationary weights are zero-padded
block-diagonal [Cin*(G+2), Cout*G], one per (ky, kx) tap, accumulated over the
9 taps in PSUM; all 9 share the same moving slice modulo the kx column offset.
conv1 (Cin=1) uses the cheaper 3-tap variant with rows (ky, g).

Feature planes live in DRAM in a channel-interleaved layout [row, C, Wp] with
output partitions ordered (g, co), which makes each layer's entire output a
single DMA write. All matmuls run in bf16.

Attention: logits S^T = K_c^T Q_j into a 3-bank PSUM tile (3 key-chunks), one
wide exp on the Scalar engine per 3 chunks, then the relative-position bias is
applied multiplicatively on the Vector engine: exp(s+b) = exp(s) * EB where
EB = exp(bias atlas) is computed once on-chip. AV accumulates in PSUM with an
extra ones-row in V giving the softmax denominators; the final divide uses a
gpsimd partition_broadcast of the reciprocal row.
"""
import sys

sys.path.insert(0, '/opt/trn_rl_repo')

import numpy as np

try:
    from ml_dtypes import bfloat16 as BF16_NP
except ImportError:  # pragma: no cover
    import jax.numpy as _jnp
    BF16_NP = _jnp.bfloat16

NUM_HEADS = 4
DIM_HEAD = 64
TABLE_M = 160
B = 2
N = 4096          # tokens (64 x 64)
GRID = 64
NCH = 32          # m-chunks of 128 keys
NQC = 8           # n-chunks of 512 queries
CF = 8192         # 127 * 64 compact-table window length

# conv layer configs: (Cin, Cout, H, W, G rows-per-group); G divides H
# layer 0 uses the 3-tap scheme (rows 3*G), layers 1+ the 9-tap (Cin*(G+2))
CONVS = [(1, 3, 256, 256, 32), (3, 3, 256, 256, 32),
         (3, 6, 128, 128, 16), (6, 6, 128, 128, 16),
         (6, 9, 64, 64, 8), (9, 9, 64, 64, 8)]

# plane name -> (C, H, W); layout is channel-interleaved [row, C, Wp]
PLANES = {
    'x':  (1, 256, 256),
    'm1': (3, 256, 256),
    'm2': (3, 256, 256),
    'p1': (3, 128, 128),
    'm3': (6, 128, 128),
    'm4': (6, 128, 128),
    'p2': (6, 64, 64),
    'm5': (9, 64, 64),
}


def _lay(C, H, W):
    Wp = W + 2
    Goff = Wp + 1
    CWp = C * Wp
    return Wp, Goff, CWp, (H + 2) * CWp + 2 * Goff  # Wp, guard, row stride, len


_BUILD_CACHE = {}


def _build():
    if 'nc' in _BUILD_CACHE:
        return _BUILD_CACHE['nc']
    import concourse.bass as bass
    import concourse.mybir as mybir
    import concourse.tile as tile
    from concourse import bacc

    F32 = mybir.dt.float32
    BF16 = mybir.dt.bfloat16
    AF = mybir.ActivationFunctionType
    ALU = mybir.AluOpType

    nc = bacc.Bacc("TRN2", target_bir_lowering=False, debug=False, num_devices=8)

    # ---- external inputs (per-core shards prepared on host) ----
    _, _, _, BLX = _lay(*PLANES['x'])
    x_d = nc.dram_tensor("x", [1, BLX], BF16, kind="ExternalInput")
    wk_d, bx_d = [], []
    for i, (ci, co, _, _, G) in enumerate(CONVS):
        if i == 0:
            rows, cols, ntap = 3 * G, co * G, 3
        else:
            rows, cols, ntap = ci * (G + 2), co * G, 9
        wk_d.append(nc.dram_tensor(f"w{i}", [rows, ntap * cols], BF16,
                                   kind="ExternalInput"))
        bx_d.append(nc.dram_tensor(f"bx{i}", [cols], F32, kind="ExternalInput"))
    wqkv_d = nc.dram_tensor("wqkv", [9, 192], BF16, kind="ExternalInput")
    wa_d = nc.dram_tensor("watlas", [128, CF], BF16, kind="ExternalInput")
    out_d = nc.dram_tensor("out", [64, N], F32, kind="ExternalOutput")

    with tile.TileContext(nc) as tc:
        with tc.tile_pool(name="const", bufs=1) as const, \
             tc.tile_pool(name="work", bufs=2) as work, \
             tc.tile_pool(name="dram", bufs=1, space="DRAM") as dram:

            # ---------------- constant tiles ----------------
            wkt, bxt = [], []
            for i, (ci, co, _, _, G) in enumerate(CONVS):
                if i == 0:
                    rows, cols, ntap = 3 * G, co * G, 3
                else:
                    rows, cols, ntap = ci * (G + 2), co * G, 9
                wkt.append(const.tile([rows, ntap, cols], BF16, tag=f"w{i}",
                                      name=f"w{i}"))
                bxt.append(const.tile([cols, 1], F32, tag=f"bx{i}",
                                      name=f"bx{i}"))

            zeros_bf = const.tile([128, 512], BF16, tag="zbf")
            nc.vector.memset(zeros_bf, 0.0)

            # prefetch all constants on the scalar queue (layer order) so the
            # sync queue only carries the latency-critical mov loads
            for i in range(6):
                nc.scalar.dma_start(
                    out=wkt[i], in_=wk_d[i].rearrange(
                        "r (t c) -> r t c", t=wkt[i].shape[1]))
                nc.scalar.dma_start(out=bxt[i], in_=bx_d[i][:, None])
            wqkv_t = const.tile([9, 192], BF16, tag="wqkv")
            nc.scalar.dma_start(out=wqkv_t, in_=wqkv_d[:, :])
            Wt = const.tile([128, CF], BF16, tag="W")
            nc.scalar.dma_start(out=Wt, in_=wa_d[:, :])
            EB = const.tile([128, CF], BF16, tag="EB")
            nc.scalar.activation(out=EB, in_=Wt, func=AF.Exp)

            # ---------------- DRAM planes + zero scratch ----------------
            geom, plane = {}, {}
            for nm, (C, H, Wd) in PLANES.items():
                geom[nm] = _lay(C, H, Wd)
                if nm != 'x':
                    plane[nm] = dram.tile([1, geom[nm][3]], BF16, tag=nm,
                                          name=nm)
            tok_d = dram.tile([1, 9 * N], BF16, tag="tok", name="tok")
            zs = dram.tile([1, 8192], BF16, tag="zs", name="zs")
            nc.sync.dma_start(
                out=bass.AP(tensor=zs.tensor, offset=0, ap=[[64, 128], [1, 64]]),
                in_=zeros_bf[0:128, 0:64])

            def zfill(tensor, offset, ap):
                total = 1
                for _, cnt in ap:
                    total *= cnt
                assert total <= 8192, total
                nc.sync.dma_start(
                    out=bass.AP(tensor=tensor, offset=offset, ap=ap),
                    in_=bass.AP(tensor=zs.tensor, offset=0, ap=[[1, total]]))

            # guards: top row + bottom guard row for every plane; pool-output
            # planes also need column guards (pools write only W2 columns)
            for nm in ['m1', 'm2', 'p1', 'm3', 'm4', 'p2', 'm5']:
                C, H, Wd = PLANES[nm]
                Wp, Goff, CWp, bl = geom[nm]
                t = plane[nm].tensor
                zfill(t, 0, [[1, Goff + CWp]])
                zfill(t, Goff + (H + 1) * CWp, [[1, CWp + Goff]])
                if nm in ('p1', 'p2'):
                    zfill(t, Goff + CWp, [[Wp, H * C]])
                    zfill(t, Goff + CWp + Wp - 1, [[Wp, H * C]])

            # ---------------- conv backbone ----------------
            def conv_layer(li, in_nm, out_nm):
                Cin, Cout, H, Wd, G = CONVS[li]
                Wp, Goff, CWp, bl = geom[in_nm]
                Sf = H // G
                mode3 = (li == 0)
                rows = 3 * G if mode3 else Cin * (G + 2)
                cols = Cout * G
                in_t = x_d if in_nm == 'x' else plane[in_nm].tensor

                mov = work.tile([rows, Sf, Wp], BF16, tag="mov", bufs=2,
                                name=f"mov{li}")
                if mode3:
                    for ky in range(3):
                        src = bass.AP(tensor=in_t, offset=Goff + ky * Wp,
                                      ap=[[Wp, G], [G * Wp, Sf], [1, Wp]])
                        nc.sync.dma_start(
                            out=mov[ky * G:(ky + 1) * G, :, :], in_=src)
                else:
                    for ci in range(Cin):
                        src = bass.AP(tensor=in_t, offset=Goff + ci * Wp,
                                      ap=[[CWp, G + 2], [G * CWp, Sf], [1, Wp]])
                        pb = ci * (G + 2)
                        nc.sync.dma_start(
                            out=mov[pb:pb + G + 2, :, :], in_=src)

                Wdo = Wd + 2  # write padded width so edge guards ride along
                out_t = work.tile([cols, Sf, Wdo], BF16, tag="out", bufs=2,
                                  name=f"out{li}")
                nc.vector.memset(out_t[:, :, 0:1], 0.0)
                nc.vector.memset(out_t[:, :, Wdo - 1:Wdo], 0.0)

                ns_max = 512 // Wd
                chs = [(s, min(ns_max, Sf - s)) for s in range(0, Sf, ns_max)]
                for g0 in range(0, len(chs), 3):
                    grp = chs[g0:g0 + 3]
                    pts = [psc.tile([cols, 512], F32, tag="cps",
                                    name=f"cps{li}_{g0}_{gi}")
                           for gi in range(len(grp))]
                    ntap = 3 if mode3 else 9
                    for t in range(ntap):
                        kx = t if mode3 else t % 3
                        for pt, (s0, ns) in zip(pts, grp):
                            nc.tensor.matmul(
                                pt[:, 0:ns * Wd], wkt[li][:, t, :],
                                mov[:, s0:s0 + ns, kx:kx + Wd],
                                start=(t == 0), stop=(t == ntap - 1))
                    for pt, (s0, ns) in zip(pts, grp):
                        nc.vector.scalar_tensor_tensor(
                            out=out_t[:, s0:s0 + ns, 1:1 + Wd],
                            in0=pt[:, 0:ns * Wd],
                            scalar=bxt[li], in1=zeros_bf[0:cols, 0:ns * Wd],
                            op0=ALU.add, op1=ALU.max)

                if out_nm == 'tok':
                    # partitions (co, g): addr = s*Cout*G*64 + (co*G+g)*64 + x
                    dst = bass.AP(tensor=tok_d.tensor, offset=0,
                                  ap=[[64, cols], [cols * 64, Sf], [1, 64]])
                    nc.gpsimd.dma_start(out=dst, in_=out_t[:, :, 1:1 + Wd])
                    return
                # partitions (g, co): addr = Goff + (s*G+g+1)*Cout*Wpo + co*Wpo
                Wpo, Goffo, CWpo, blo = geom[out_nm]
                dst = bass.AP(tensor=plane[out_nm].tensor,
                              offset=Goffo + CWpo,
                              ap=[[Wpo, cols], [G * CWpo, Sf], [1, Wdo]])
                nc.gpsimd.dma_start(out=dst, in_=out_t)

            def pool_layer(in_nm, out_nm):
                C, H, Wd = PLANES[in_nm]
                Wp, Goff, CWp, bl = geom[in_nm]
                H2, W2 = H // 2, Wd // 2
                Wp2, Goff2, CWp2, bl2 = geom[out_nm]
                it, ot = plane[in_nm].tensor, plane[out_nm].tensor
                # one load/store per channel: output rows on partitions (H2<=128)
                for c in range(C):
                    t3 = work.tile([128, 2, Wd], BF16, tag="pool", bufs=3,
                                   name=f"pool_{in_nm}_{c}")
                    src = bass.AP(tensor=it,
                                  offset=Goff + CWp + c * Wp + 1,
                                  ap=[[2 * CWp, H2], [CWp, 2], [1, Wd]])
                    (nc.sync if c % 2 else nc.scalar).dma_start(
                        out=t3[0:H2, :, :], in_=src)
                    m1t = work.tile([128, 2, W2], BF16, tag="plw", bufs=2)
                    nc.vector.tensor_max(m1t[0:H2], t3[0:H2, :, 0::2],
                                         t3[0:H2, :, 1::2])
                    m2t = work.tile([128, W2], BF16, tag="plh", bufs=2)
                    nc.vector.tensor_max(m2t[0:H2], m1t[0:H2, 0, :],
                                         m1t[0:H2, 1, :])
                    dst = bass.AP(tensor=ot,
                                  offset=Goff2 + CWp2 + c * Wp2 + 1,
                                  ap=[[CWp2, H2], [1, W2]])
                    (nc.gpsimd if c % 2 else nc.sync).dma_start(
                        out=dst, in_=m2t[0:H2, :])

            scope_conv = nc.named_scope("conv"); scope_conv.__enter__()
            with tc.tile_pool(name="psc", bufs=6, space="PSUM") as psc:
                conv_layer(0, 'x', 'm1')
                conv_layer(1, 'm1', 'm2')
                pool_layer('m2', 'p1')
                conv_layer(2, 'p1', 'm3')
                conv_layer(3, 'm3', 'm4')
                pool_layer('m4', 'p2')
                conv_layer(4, 'p2', 'm5')
                conv_layer(5, 'm5', 'tok')
            scope_conv.__exit__(None, None, None)

            # ---------------- tokens + q/k/v ----------------
            scope_qkv = nc.named_scope("qkv"); scope_qkv.__enter__()
            G6 = CONVS[5][4]
            tokT = const.tile([9, N], BF16, tag="tok")
            nc.sync.dma_start(
                out=tokT,
                in_=bass.AP(tensor=tok_d.tensor, offset=0,
                            ap=[[G6 * 64, 9], [9 * G6 * 64, 64 // G6],
                                [1, G6 * 64]]))

            qT = const.tile([64, N], BF16, tag="qT")
            kT = const.tile([64, N], BF16, tag="kT")
            v_sb = const.tile([128, NCH, 65], BF16, tag="v")
            nc.vector.memset(v_sb, 1.0)

            with tc.tile_pool(name="psq", bufs=2, space="PSUM") as psq:
                for j in range(NQC):
                    ps_q = psq.tile([64, 512], F32, tag="qps")
                    nc.tensor.matmul(ps_q, wqkv_t[:, 0:64],
                                     tokT[:, j * 512:(j + 1) * 512],
                                     start=True, stop=True)
                    nc.scalar.activation(out=qT[:, j * 512:(j + 1) * 512],
                                         in_=ps_q, func=AF.Copy,
                                         scale=float(DIM_HEAD) ** -0.5)
                    ps_k = psq.tile([64, 512], F32, tag="kps")
                    nc.tensor.matmul(ps_k, wqkv_t[:, 64:128],
                                     tokT[:, j * 512:(j + 1) * 512],
                                     start=True, stop=True)
                    nc.scalar.activation(out=kT[:, j * 512:(j + 1) * 512],
                                         in_=ps_k, func=AF.Copy)
                for c in range(NCH):
                    ps_v = psq.tile([128, 64], F32, tag="vps")
                    nc.tensor.matmul(ps_v, tokT[:, c * 128:(c + 1) * 128],
                                     wqkv_t[:, 128:192], start=True, stop=True)
                    nc.vector.tensor_copy(v_sb[:, c, 0:64], ps_v)
            scope_qkv.__exit__(None, None, None)

            # ---------------- attention ----------------
            scope_attn = nc.named_scope("attn"); scope_attn.__enter__()
            with tc.tile_pool(name="pss", bufs=2, space="PSUM") as pss, \
                 tc.tile_pool(name="psa", bufs=2, space="PSUM") as psa:
                for j in range(NQC):
                    acc = psa.tile([65, 512], F32, tag="acc")
                    for cg in range(0, NCH, 3):
                        w = min(3, NCH - cg)
                        # S^T for chunks cg..cg+w-1, one 3-bank PSUM tile;
                        # exp over all w*512 columns in a single ACT op
                        s3 = pss.tile([128, 3, 512], F32, tag="s3")
                        for i in range(w):
                            c = cg + i
                            nc.tensor.matmul(s3[:, i, :],
                                             kT[:, c * 128:(c + 1) * 128],
                                             qT[:, j * 512:(j + 1) * 512],
                                             start=True, stop=True)
                        at3 = work.tile([128, 3, 512], BF16, tag="at", bufs=3)
                        nc.scalar.activation(out=at3[:, 0:w, :],
                                             in_=s3[:, 0:w, :], func=AF.Exp)
                        atb3 = work.tile([128, 3, 512], BF16, tag="atb", bufs=3)
                        for i in range(w):
                            c = cg + i
                            s0 = (8 * j - 2 * c + 63) * 64
                            nc.vector.tensor_mul(atb3[:, i, :], at3[:, i, :],
                                                 EB[:, s0:s0 + 512])
                        for i in range(w):
                            c = cg + i
                            nc.tensor.matmul(acc, v_sb[:, c, :], atb3[:, i, :],
                                             start=(c == 0),
                                             stop=(c == NCH - 1))
                    # epilogue: divide by the attention sums (row 64 of acc)
                    sums = work.tile([1, 512], F32, tag="sums", bufs=2)
                    nc.vector.tensor_copy(sums, acc[64:65, :])
                    rcp_f = work.tile([1, 512], F32, tag="rcpf", bufs=2)
                    nc.vector.reciprocal_approx_fast(out=rcp_f, in_=sums)
                    bc_sb = work.tile([64, 512], F32, tag="bcs", bufs=2)
                    nc.gpsimd.partition_broadcast(bc_sb, rcp_f)
                    res = work.tile([64, 512], F32, tag="res", bufs=2)
                    nc.vector.tensor_mul(res, acc[0:64, :], bc_sb)
                    nc.sync.dma_start(out=out_d[:, j * 512:(j + 1) * 512],
                                      in_=res)
            scope_attn.__exit__(None, None, None)

    nc.finalize()
    _BUILD_CACHE['nc'] = nc
    return nc


def _prep_inputs(inputs):
    """Build the 8 per-core input maps (layout/packing only)."""
    x = np.asarray(inputs['x'], dtype=np.float32)
    qkv_w = np.asarray(inputs['qkv_w'], dtype=np.float32)
    table = np.asarray(inputs['bias_table'], dtype=np.float32)

    Wp, Goff, CWp, BLX = _lay(*PLANES['x'])
    xbufs = []
    for b in range(B):
        pad = np.zeros((258, Wp), np.float32)
        pad[1:257, 1:257] = x[b, 0]
        buf = np.zeros((1, BLX), np.float32)
        buf[0, Goff:Goff + 258 * Wp] = pad.reshape(-1)
        xbufs.append(buf.astype(BF16_NP))

    wks, bxs = [], []
    for i, (Cin, Cout, _, _, G) in enumerate(CONVS):
        w = np.asarray(inputs[f'conv{i + 1}_w'], dtype=np.float32)
        bias = np.asarray(inputs[f'conv{i + 1}_b'], dtype=np.float32)
        ar = np.arange(G)
        last = (i == 5)
        taps = []
        if i == 0:
            for kx in range(3):
                Wk = np.zeros((3 * G, Cout * G), np.float32)
                for ky in range(3):
                    for co in range(Cout):
                        Wk[ky * G + ar, ar * Cout + co] = w[co, 0, ky, kx]
                taps.append(Wk)
        else:
            for ky in range(3):
                for kx in range(3):
                    Wk = np.zeros((Cin * (G + 2), Cout * G), np.float32)
                    for ci in range(Cin):
                        for co in range(Cout):
                            col = co * G + ar if last else ar * Cout + co
                            Wk[ci * (G + 2) + ky + ar, col] = w[co, ci, ky, kx]
                    taps.append(Wk)
        wks.append(np.concatenate(taps, axis=1).astype(BF16_NP))
        bxs.append((np.repeat(bias, G) if last
                    else np.tile(bias, G)).astype(np.float32))

    atlases = []
    for h in range(NUM_HEADS):
        tab = table[:, h].reshape(2 * TABLE_M - 1, 2 * TABLE_M - 1)
        Ct = tab[96:96 + 127, 96:96 + 127]  # [127, 127]
        tmp = np.zeros((127, 128), np.float32)
        tmp[:, :127] = Ct
        cfbuf = np.zeros(191 + 16256 + 129, np.float32)
        cfbuf[191:191 + 16256] = tmp.reshape(-1)
        sw = np.lib.stride_tricks.sliding_window_view(cfbuf, 16256)
        p = np.arange(128)
        offs = 254 - (p % 64) - 128 * (p // 64)
        full = sw[offs]                                   # [128, 127*128]
        a2 = full.reshape(128, 127, 128)[:, :, 0:64].reshape(128, 127 * 64)
        atl = np.zeros((128, CF), np.float32)
        atl[:, :127 * 64] = a2
        atlases.append(atl.astype(BF16_NP))

    in_maps = []
    for core in range(8):
        b, h = core // 4, core % 4
        m = {"x": xbufs[b], "watlas": atlases[h]}
        for i in range(6):
            m[f"w{i}"] = wks[i]
            m[f"bx{i}"] = bxs[i]
        wq = qkv_w[h * 64:(h + 1) * 64, :].T
        wk = qkv_w[256 + h * 64:256 + (h + 1) * 64, :].T
        wv = qkv_w[512 + h * 64:512 + (h + 1) * 64, :].T
        m["wqkv"] = np.ascontiguousarray(
            np.concatenate([wq, wk, wv], axis=1)).astype(BF16_NP)
        in_maps.append(m)
    return in_maps


def kernel(_trace=False, **inputs):
    from concourse.bass_utils import run_bass_kernel_spmd
    nc = _build()
    in_maps = _prep_inputs(inputs)
    import os
    tdir = os.environ.get("KTRACE_DIR")
    if tdir:
        os.makedirs(tdir, exist_ok=True)
    res = run_bass_kernel_spmd(nc, in_maps, core_ids=list(range(8)),
                               trace=_trace, tmpdir=tdir)
    if _trace:
        kernel.last_exec_ns = res.exec_time_ns
        kernel.last_results = res
    # assemble: core -> (b, h): [64(d), 4096(n)]
    O = np.stack([np.stack([res.results[b * 4 + h]["out"] for h in range(4)])
                  for b in range(B)])                      # [B, H, 64, N]
    out = O.transpose(0, 3, 1, 2).reshape(B, N, NUM_HEADS * DIM_HEAD)
    out = out.reshape(B, GRID, GRID, NUM_HEADS * DIM_HEAD)
    shift = int(np.asarray(inputs['window_size'])) // 2
    out = np.roll(out, shift=(-shift, -shift), axis=(1, 2))
    return out.astype(np.float32)
